# revision 1
# baseline (speedup 1.0000x reference)
"""BinarizedFCLayer forward on 8 trn2 NeuronCores.

    out = X @ sign(W).T      X: [8192, 2048] f32, W: [2048, 2048] f32
                             sign(w) = +1 if w >= 0 else -1

Strategy
--------
Data-parallel over the batch dim of X: core c computes rows
[c*1024, (c+1)*1024) of the output; W is replicated.

Per core (M=1024, K=2048, N=2048 -> 512 matmuls of N=512 ~= 110.9 us PE
at the warm 2.4 GHz issue rate; 24 MiB of f32 input reads ~= 70.3 us at
the 358 GB/s per-core HBM limit):
  * TensorE contracts over the partition dim, so both operands carry K on
    partitions. The host passes X^T shards and W^T (pure layout prep).
  * X^T: SWDGE cast-DMA f32->fp16 into a resident tile (fp16 keeps 11
    mantissa bits -> output rel err ~2e-4 vs the fp32 reference).
  * W^T: SWDGE cast-DMA f32->bf16 (bf16 keeps the f32 exponent, so
    sign(bf16(w)) == sign(w)); ONE DVE pass binarizes to +-0.5 fp16
    ((w >= 0) - 0.5); the missing x2 is folded into the PSUM->SBUF
    activation copy (scale=2.0, exact power-of-two).
  * Input DMA pieces are issued as the first gpsimd instructions in an
    order chosen by an offline delivery-vs-consumption search: fine W/X
    pieces interleaved ~1:1 by bytes so unlocked matmul work stays ahead
    of the PE once it starts real work (~19 us in).
  * PE: units of (n-chunk, m-quarter) = two PSUM banks alternating
    every matmul (mo=0,1 share one rhs slice) so a bank's drain always
    overlaps the other's fill -> the steady 216 ns/matmul issue rate;
    up to 4 units (8 banks) in flight. The matmul stream follows the
    same offline schedule, keeping mid-stream stalls ~1 us so the HAM
    clock gate rarely re-throttles. Warm-up matmuls bridge the DMA
    prologue so real matmuls start at the full 2.4 GHz rate.
  * Outputs are buffered in 23 resident SBUF tiles (PSUM->SBUF copies
    on ACT with scale=2.0; DVE for late units) and stored as bf16 (half the
    write traffic) from the otherwise-idle sync HWDGE queue behind a gate DMA that reads the
    last input piece -- store writes mostly stay off the HBM read
    stream, which would otherwise lose ~30% of input bandwidth.

The walrus build here allows at most ONE sync wait per instruction, so a
post-pass splits any multi-wait instruction into single-wait NoOps on the
same engine placed immediately before it.
"""

import numpy as np

try:
    import concourse.bass as bass
except ImportError:  # harness may run from a bare directory
    import sys
    for p in ("/opt/trn_rl_repo", "/root/.axon_site/_ro/trn_rl_repo"):
        if p not in sys.path:
            sys.path.append(p)
    import concourse.bass as bass

import concourse.mybir as mybir
from concourse.tile import TileContext
from concourse.bass_utils import run_bass_kernel_spmd

P = 128
N_CORES = 8
M_FULL, K, N = 8192, 2048, 2048
M = M_FULL // N_CORES          # 1024 rows of X per core
KT = K // P                    # 16 k-tiles
MT = M // P                    # 8 m-tiles of 128
NCH, NW = 4, 512               # 4 n-chunks of 512 (one PSUM bank each)
N_WARM = 185                   # dummy matmuls bridging preamble -> first data

f32 = mybir.dt.float32
f16 = mybir.dt.float16
bf16 = mybir.dt.bfloat16

# Input DMA piece order, from an offline search over delivery schedules
# (W nn, kt_lo, kt_hi) covers wt3[:, kt_lo:kt_hi, nn*512:(nn+1)*512];
# (X mq, kt_lo, kt_hi) covers xt3[:, kt_lo:kt_hi, mq*256:(mq+1)*256).
PIECE_ORDER = [
    ('W', 3, 0, 4),
    ('W', 3, 4, 8),
    ('W', 2, 0, 4),
    ('X', 0, 0, 2),
    ('X', 3, 0, 8),
    ('X', 1, 0, 8),
    ('X', 0, 2, 4),
    ('X', 3, 8, 16),
    ('X', 0, 4, 8),
    ('W', 3, 8, 12),
    ('W', 3, 12, 16),
    ('X', 1, 8, 16),
    ('X', 2, 0, 8),
    ('W', 2, 4, 8),
    ('W', 0, 0, 2),
    ('W', 2, 8, 12),
    ('W', 1, 0, 4),
    ('X', 0, 8, 16),
    ('W', 1, 4, 8),
    ('W', 0, 2, 4),
    ('W', 0, 4, 8),
    ('W', 1, 8, 12),
    ('W', 2, 12, 16),
    ('W', 0, 8, 12),
    ('W', 1, 12, 16),
    ('X', 2, 8, 16),
    ('W', 0, 12, 16),
]

# Delivery/consumption model used to derive the static matmul order
# (calibrated against HW traces of this kernel).
EMIT_T0 = 9.4         # us: Q7 emission start + SWDGE first-byte latency
EMIT_US = 1.0        # us of serial Q7 emission per dma_start
READ_RATE = 2.65      # us per MiB of HBM reads, no store overlap
RECEIPT = 1.2         # us from last byte to semaphore fire (HBM receipt)
BIN_PER_MIB = 0.55    # us DVE binarize per MiB(read) of W piece
BIN_FIX = 0.12
PE_T0 = 7.7           # us: PE queue starts draining
MM_US = 0.2165        # warm matmul issue period at N=512


def _piece_mib(p):
    kind, _, klo, khi = p
    return P * (khi - klo) * (512 if kind == 'W' else 256) * 4 / (1 << 20)


def _derive_exec_order():
    """Greedy consumption of the modeled delivery timeline -> static
    (nn, mq, kt) schedule; each step is a PSUM-bank-alternating pair of
    matmuls (mo=0,1) so drain always overlaps the next fill. At most 4
    pairs (8 banks) in flight."""
    avail = {}
    t = EMIT_T0
    emit_end = EMIT_T0
    for p in PIECE_ORDER:
        emit_end += EMIT_US
        start = max(t, emit_end)
        t = start + _piece_mib(p) * READ_RATE
        avail[p] = t + RECEIPT + (BIN_FIX + _piece_mib(p) * BIN_PER_MIB
                                  if p[0] == 'W' else 0.0)
    R = {}
    for nn in range(NCH):
        for mq in range(4):
            for kt in range(KT):
                w = min(ta for q, ta in avail.items()
                        if q[0] == 'W' and q[1] == nn and q[2] <= kt < q[3])
                x = min(ta for q, ta in avail.items()
                        if q[0] == 'X' and q[1] == mq and q[2] <= kt < q[3])
                R[(nn, mq, kt)] = max(w, x)
    units = [(nn, mq) for nn in range(NCH) for mq in range(4)]
    unit_order = sorted(
        units, key=lambda u: (max(R[(u[0], u[1], kt)] for kt in range(KT)),
                              R[(u[0], u[1], 0)]))
    ptr = {u: 0 for u in units}
    done = {u: False for u in units}
    open_units = []
    t = PE_T0
    order = []
    while len(order) < len(units) * KT:
        best, best_r = None, None
        for u in unit_order:
            if done[u]:
                continue
            if u not in open_units and len(open_units) >= 4:
                continue
            r = R[(u[0], u[1], ptr[u])]
            if r <= t:
                best = u
                break
            if best_r is None or r < best_r:
                best_r, best = r, u
        u = best
        t = max(t, R[(u[0], u[1], ptr[u])]) + 2 * MM_US
        if u not in open_units:
            open_units.append(u)
        order.append((u[0], u[1], ptr[u]))
        ptr[u] += 1
        if ptr[u] == KT:
            done[u] = True
            open_units.remove(u)
    return order


def _split_multiwait_instructions(nc: bass.Bass) -> int:
    """walrus codegen rejects >1 sync wait per instruction. Hoist extra waits
    onto fresh single-wait NoOps on the same engine right before the
    offending instruction (same-engine sequential waits are equivalent)."""
    n_split = 0
    for fn in nc.m.functions:
        for blk in fn.blocks:
            out = []
            for inst in blk.instructions:
                si = inst.sync_info
                if si is not None and si.on_wait and len(si.on_wait) > 1:
                    waits = list(si.on_wait)
                    for j, w in enumerate(waits[:-1]):
                        nop = mybir.InstNoOp(
                            name=f"{inst.name}_wsplit{j}", ins=[], outs=[])
                        nop.engine = inst.engine
                        nop.sync_info = mybir.SyncInfo(
                            on_wait=[w], on_update=[])
                        out.append(nop)
                        n_split += 1
                    inst.sync_info = mybir.SyncInfo(
                        on_wait=[waits[-1]],
                        on_update=list(si.on_update or []))
                out.append(inst)
            blk.instructions[:] = out
    return n_split


def _build_nc() -> bass.Bass:
    exec_order = _derive_exec_order()

    nc = bass.Bass()
    xt = nc.declare_dram_parameter("xt", [K, M], f32, isOutput=False)
    wt = nc.declare_dram_parameter("wt", [K, N], f32, isOutput=False)
    out = nc.declare_dram_parameter("out", [M, N], bf16, isOutput=True)

    xt3 = xt[:].rearrange("(kt p) m -> p kt m", p=P)    # [128, 16, 1024]
    wt3 = wt[:].rearrange("(kt p) n -> p kt n", p=P)    # [128, 16, 2048]
    out3 = out[:].rearrange("(mt p) n -> p mt n", p=P)  # [128, 8, 2048]

    with TileContext(nc) as tc:
        with (
            tc.tile_pool(name="resident", bufs=1) as res_pool,
            tc.tile_pool(name="wq", bufs=4) as wq_pool,
            tc.tile_pool(name="osb", bufs=23) as o_pool,
            tc.tile_pool(name="gate", bufs=1) as g_pool,
            tc.tile_pool(name="psum", bufs=8, space="PSUM") as p_pool,
            tc.tile_pool(name="warm", bufs=1) as warm_pool,
        ):
            xq = res_pool.tile([P, KT, M], f16, tag="xq", name="xq")
            wraw = res_pool.tile([P, KT, N], bf16, tag="wraw", name="wraw")
            wqs = [wq_pool.tile([P, KT, NW], f16, tag="wq", name=f"wq{nn}")
                   for nn in range(NCH)]

            # PE warm-up first in each queue: memset leads the DVE queue
            # (so it doesn't sit behind the binarizes in DVE FIFO order),
            # dummy matmuls lead the PE queue, bridging the DMA prologue
            # and holding the HAM clock gate at 8/8 for the real stream.
            wsrc = warm_pool.tile([P, P], f16, tag="wsrc", name="wsrc")
            nc.vector.memset(wsrc[:], 0.0)
            wps = p_pool.tile([P, NW], f32, tag="ps", name="wps")
            for _ in range(N_WARM):
                nc.tensor.matmul(wps[:, :P], lhsT=wsrc[:], rhs=wsrc[:],
                                 start=True, stop=True)

            # Input pieces: SWDGE cast-DMAs lead the gpsimd queue so it
            # starts streaming immediately; each W piece is binarized on
            # DVE the moment it lands (single pass, (w >= 0) - 0.5 ->
            # +-0.5 in fp16).
            for pi, (kind, idx, klo, khi) in enumerate(PIECE_ORDER):
                ks = slice(klo, khi)
                if kind == 'W':
                    ns = slice(idx * NW, (idx + 1) * NW)
                    pdma = nc.gpsimd.dma_start(out=wraw[:, ks, ns],
                                               in_=wt3[:, ks, ns])
                    nc.vector.tensor_scalar(
                        wqs[idx][:, ks, :], wraw[:, ks, ns], 0.0, 0.5,
                        mybir.AluOpType.is_ge, mybir.AluOpType.subtract)
                else:
                    ms = slice(idx * 256, (idx + 1) * 256)
                    pdma = nc.gpsimd.dma_start(out=xq[:, ks, ms],
                                               in_=xt3[:, ks, ms])

            # Store gate: a tiny sync-queue DMA that reads the tail of
            # the last input piece. Output stores mostly queue behind it
            # in sync order, so their HBM writes largely stay off the
            # input read stream; the buffered outputs burst out during
            # the final stretch of pure compute.
            lk, li, lklo, lkhi = PIECE_ORDER[-1]
            gsc = g_pool.tile([1, 4], f16 if lk == 'X' else bf16,
                              tag="gate", name="gate")
            gsrc = xq if lk == 'X' else wraw
            gcol = (li + 1) * (256 if lk == 'X' else NW)
            nc.sync.dma_start(
                out=gsc[:],
                in_=gsrc[0:1, lkhi - 1:lkhi, gcol - 4:gcol])

            # Real matmuls in the scheduled order. Unit (nn, mq) = two
            # PSUM banks (mo=0,1) alternating every matmul so the PE
            # drain of one bank overlaps the fill of the other; both
            # matmuls of a step share the same rhs slice. On a unit's
            # last step each bank is copied to SBUF (ACT, scale=2.0
            # completes the binarization) and stored from the idle sync
            # HWDGE queue as a 0.25 MiB DMA.
            unit_psums = {}
            n_done = 0
            for nn, mq, kt in exec_order:
                u = (nn, mq)
                if kt == 0:
                    unit_psums[u] = [
                        p_pool.tile([P, NW], f32, tag="ps",
                                    name=f"ps{nn}_{mq}_{mo}")
                        for mo in range(2)
                    ]
                for mo in range(2):
                    mcol = mq * 256 + mo * P
                    nc.tensor.matmul(
                        unit_psums[u][mo][:],
                        lhsT=xq[:, kt, mcol:mcol + P],
                        rhs=wqs[nn][:, kt, :],
                        start=(kt == 0),
                        stop=(kt == KT - 1),
                    )
                if kt == KT - 1:
                    n_done += 1
                    for mo in range(2):
                        osb = o_pool.tile([P, NW], bf16, tag="osb",
                                          name=f"osb{nn}_{mq}_{mo}")
                        if mo == 1 and n_done > 12:
                            # late units: x2 copy on DVE (long idle by now)
                            # so it runs parallel with mo0's ACT copy and
                            # the final store issues ~0.7 us sooner.
                            nc.vector.tensor_scalar(
                                osb[:], unit_psums[u][mo][:], 2.0, None,
                                mybir.AluOpType.mult)
                        else:
                            nc.scalar.activation(
                                out=osb[:], in_=unit_psums[u][mo][:],
                                func=mybir.ActivationFunctionType.Copy,
                                scale=2.0)
                        nc.sync.dma_start(
                            out=out3[:, mq * 2 + mo, nn * NW:(nn + 1) * NW],
                            in_=osb[:])

    _split_multiwait_instructions(nc)
    return nc


_NC_CACHE = None


def _get_nc() -> bass.Bass:
    global _NC_CACHE
    if _NC_CACHE is None:
        _NC_CACHE = _build_nc()
    return _NC_CACHE


def _run(inputs: dict, trace: bool = False, **kw):
    X = np.asarray(inputs["X"], dtype=np.float32)
    W = np.asarray(inputs["W"], dtype=np.float32)
    assert X.shape == (M_FULL, K) and W.shape == (N, K)

    XT = np.ascontiguousarray(X.T)            # [K, M_FULL]
    WT = np.ascontiguousarray(W.T)            # [K, N]
    in_maps = [
        {"xt": np.ascontiguousarray(XT[:, c * M:(c + 1) * M]), "wt": WT}
        for c in range(N_CORES)
    ]
    res = run_bass_kernel_spmd(
        _get_nc(), in_maps, list(range(N_CORES)), trace=trace, **kw)
    out = np.concatenate([res.results[c]["out"] for c in range(N_CORES)],
                         axis=0).astype(np.float32)
    return out, res


def kernel(X: np.ndarray, W: np.ndarray) -> np.ndarray:
    out, _ = _run({"X": X, "W": W})
    return out



# revision 2
# speedup vs baseline: 1.2025x; 1.2025x over previous
"""BinarizedFCLayer forward on 8 trn2 NeuronCores.

    out = X @ sign(W).T      X: [8192, 2048] f32, W: [2048, 2048] f32
                             sign(w) = +1 if w >= 0 else -1

Strategy
--------
Data-parallel over the batch dim of X: core c computes rows
[c*1024, (c+1)*1024) of the output; W is replicated.

Per core (M=1024, K=2048, N=2048), W-stationary / out^T orientation:
psum[n=128, m=512] = sum_k Sq^T[k, n-tile] * X^T[k, m-half].

Mixed-precision contraction split to beat the 1-column/cycle PE limit:
  * k-tiles 0..7  (K=1024): fp16 matmuls (X cast f32->f16 in DMA; rel
    err ~2e-4).
  * k-tiles 8..15 (K=1024): fp8e4 (e4m3) matmuls in DoubleRow perf mode
    -- two k-tiles contracted per matmul at the same 512-cycle issue
    cost, i.e. 2x throughput. e4m3 X quantization costs ~2.6e-2 rel err
    on the half of K it covers -> total ~1.9e-2, inside the 2e-2 gate.
  * W is cast f32->bf16 in DMA (sign-preserving), binarized on DVE to
    +-0.5 in one pass ((w>=0)-0.5) into an fp16 copy (k 0..7) and an
    fp8 copy (k 8..15); the missing x2 is folded into the PSUM->SBUF
    ACT copy (scale=2.0).
This cuts the PE stream from 512 to 384 matmul slots of N=512.

Work units: (nt in 0..15, mh in 0..1) -> one PSUM bank each, 8 fp16 +
4 DoubleRow matmuls, ACT copy to SBUF fp16, store via the sync queue
behind a gate DMA (keeps output writes off the HBM read stream).
Unit order: mh=0 pass over nt 0..15, then mh=1 pass (DoubleRow matmuls
first inside mh=1 units -- the fp8 X half arrives before the fp16 one).
DMA piece order follows a delivery model (~0.34 MiB/us starting ~9.4 us
after kernel start); warm-up matmuls bridge the DMA prologue so the
real stream starts warm at ~23 us with data always ahead of the PE.

The walrus build allows at most ONE sync wait per instruction, so a
post-pass splits any multi-wait instruction into single-wait NoOps on
the same engine placed immediately before it.
"""

import numpy as np

try:
    import concourse.bass as bass
except ImportError:  # harness may run from a bare directory
    import sys
    for p in ("/opt/trn_rl_repo", "/root/.axon_site/_ro/trn_rl_repo"):
        if p not in sys.path:
            sys.path.append(p)
    import concourse.bass as bass

import concourse.mybir as mybir
from concourse.tile import TileContext
from concourse.bass_utils import run_bass_kernel_spmd

P = 128
N_CORES = 8
M_FULL, K, N = 8192, 2048, 2048
M = M_FULL // N_CORES          # 1024 rows of X per core
KT = K // P                    # 16 k-tiles
KT16 = 8                       # k-tiles 0..7  -> fp16 path
KT8 = 8                        # k-tiles 8..15 -> fp8 DoubleRow path
NT = N // P                    # 16 n-tiles of 128 (stationary side)
MH = 2                         # 2 m-halves of 512 (moving free dim)
MW = 512
N_WARM = 185                   # dummy matmuls bridging preamble -> first data

f32 = mybir.dt.float32
f16 = mybir.dt.float16
bf16 = mybir.dt.bfloat16
f8e4 = mybir.dt.float8e4
DR = mybir.MatmulPerfMode.DoubleRow

# Input DMA piece order (gpsimd/SWDGE queue order == transfer order).
# ('W16', nt): wraw[:, 0:8,  nt*128:(nt+1)*128]   (0.5 MiB f32 read)
# ('W8',  nt): wraw[:, 8:16, nt*128:(nt+1)*128]   (0.5 MiB)
# ('W',   nt): wraw[:, :,    nt*128:(nt+1)*128]   (1 MiB)
# ('X16', mh): xq16 half    (2 MiB)  ('X16a/b', mh): kt 0:4 / 4:8 (1 MiB)
# ('X8',  mh): xq8 half     (1 MiB)
PIECE_ORDER = (
    [('W16', 0), ('X16', 0), ('W8', 0), ('X8', 0)]
    + [('W', nt) for nt in range(1, 16)]
    + [('X8', 1), ('X16a', 1), ('X16b', 1)]
)


def _split_multiwait_instructions(nc: bass.Bass) -> int:
    """walrus codegen rejects >1 sync wait per instruction. Hoist extra waits
    onto fresh single-wait NoOps on the same engine right before the
    offending instruction (same-engine sequential waits are equivalent)."""
    n_split = 0
    for fn in nc.m.functions:
        for blk in fn.blocks:
            out = []
            for inst in blk.instructions:
                si = inst.sync_info
                if si is not None and si.on_wait and len(si.on_wait) > 1:
                    waits = list(si.on_wait)
                    for j, w in enumerate(waits[:-1]):
                        nop = mybir.InstNoOp(
                            name=f"{inst.name}_wsplit{j}", ins=[], outs=[])
                        nop.engine = inst.engine
                        nop.sync_info = mybir.SyncInfo(
                            on_wait=[w], on_update=[])
                        out.append(nop)
                        n_split += 1
                    inst.sync_info = mybir.SyncInfo(
                        on_wait=[waits[-1]],
                        on_update=list(si.on_update or []))
                out.append(inst)
            blk.instructions[:] = out
    return n_split


def _build_nc() -> bass.Bass:
    nc = bass.Bass()
    xt = nc.declare_dram_parameter("xt", [K, M], f32, isOutput=False)
    wt = nc.declare_dram_parameter("wt", [K, N], f32, isOutput=False)
    out = nc.declare_dram_parameter("out", [N, M], f16, isOutput=True)

    xt3 = xt[:].rearrange("(kt p) m -> p kt m", p=P)    # [128, 16, 1024]
    wt3 = wt[:].rearrange("(kt p) n -> p kt n", p=P)    # [128, 16, 2048]
    out3 = out[:].rearrange("(nt p) m -> p nt m", p=P)  # [128, 16, 1024]

    with TileContext(nc) as tc:
        with (
            tc.tile_pool(name="resident", bufs=1) as res_pool,
            tc.tile_pool(name="osb", bufs=32) as o_pool,
            tc.tile_pool(name="gate", bufs=1) as g_pool,
            tc.tile_pool(name="psum", bufs=8, space="PSUM") as p_pool,
            tc.tile_pool(name="warm", bufs=1) as warm_pool,
        ):
            xq16 = res_pool.tile([P, KT16, M], f16, tag="xq16", name="xq16")
            xq8 = res_pool.tile([P, KT8, M], f8e4, tag="xq8", name="xq8")
            wraw = res_pool.tile([P, KT, N], bf16, tag="wraw", name="wraw")
            wq16 = res_pool.tile([P, KT16, N], f16, tag="wq16", name="wq16")
            wq8 = res_pool.tile([P, KT8, N], f8e4, tag="wq8", name="wq8")

            # PE warm-up first in each queue: memset leads the DVE queue,
            # dummy matmuls lead the PE queue, bridging the DMA prologue
            # and holding the HAM clock gate at 8/8 for the real stream.
            wsrc = warm_pool.tile([P, P], f16, tag="wsrc", name="wsrc")
            nc.vector.memset(wsrc[:], 0.0)
            wps = p_pool.tile([P, MW], f32, tag="ps", name="wps")
            for _ in range(N_WARM):
                nc.tensor.matmul(wps[:, :P], lhsT=wsrc[:], rhs=wsrc[:],
                                 start=True, stop=True)

            # Input pieces on the SWDGE queue; every W piece is binarized
            # on DVE as soon as it lands ((w >= 0) - 0.5 -> +-0.5).
            def bin16(nt):
                ns = slice(nt * P, (nt + 1) * P)
                nc.vector.tensor_scalar(
                    wq16[:, :, ns], wraw[:, 0:KT16, ns], 0.0, 0.5,
                    mybir.AluOpType.is_ge, mybir.AluOpType.subtract)

            def bin8(nt):
                ns = slice(nt * P, (nt + 1) * P)
                nc.vector.tensor_scalar(
                    wq8[:, :, ns], wraw[:, KT16:KT, ns], 0.0, 0.5,
                    mybir.AluOpType.is_ge, mybir.AluOpType.subtract)

            for kind, idx in PIECE_ORDER:
                if kind == 'W16':
                    ns = slice(idx * P, (idx + 1) * P)
                    nc.gpsimd.dma_start(out=wraw[:, 0:KT16, ns],
                                        in_=wt3[:, 0:KT16, ns])
                    bin16(idx)
                elif kind == 'W8':
                    ns = slice(idx * P, (idx + 1) * P)
                    nc.gpsimd.dma_start(out=wraw[:, KT16:KT, ns],
                                        in_=wt3[:, KT16:KT, ns])
                    bin8(idx)
                elif kind == 'W':
                    ns = slice(idx * P, (idx + 1) * P)
                    nc.gpsimd.dma_start(out=wraw[:, :, ns],
                                        in_=wt3[:, :, ns])
                    bin16(idx)
                    bin8(idx)
                elif kind == 'X16':
                    ms = slice(idx * MW, (idx + 1) * MW)
                    nc.gpsimd.dma_start(out=xq16[:, :, ms],
                                        in_=xt3[:, 0:KT16, ms])
                elif kind == 'X16a':
                    ms = slice(idx * MW, (idx + 1) * MW)
                    nc.gpsimd.dma_start(out=xq16[:, 0:4, ms],
                                        in_=xt3[:, 0:4, ms])
                elif kind == 'X16b':
                    ms = slice(idx * MW, (idx + 1) * MW)
                    nc.gpsimd.dma_start(out=xq16[:, 4:8, ms],
                                        in_=xt3[:, 4:8, ms])
                elif kind == 'X8':
                    ms = slice(idx * MW, (idx + 1) * MW)
                    nc.gpsimd.dma_start(out=xq8[:, :, ms],
                                        in_=xt3[:, KT16:KT, ms])

            # Store gate: a tiny sync-queue DMA that reads the tail of
            # the last input piece. Output stores queue behind it in
            # sync-FIFO order, so their HBM writes stay off the input
            # read stream until all reads are done.
            gsc = g_pool.tile([1, 4], f16, tag="gate", name="gate")
            nc.sync.dma_start(
                out=gsc[:],
                in_=xq16[0:1, KT16 - 1:KT16, M - 4:M])

            # Real matmul stream: mh=0 pass over nt 0..15, then mh=1.
            for u in range(NT * MH):
                mh, nt = (0, u) if u < NT else (1, u - NT)
                ns = slice(nt * P, (nt + 1) * P)
                ms = slice(mh * MW, (mh + 1) * MW)
                ps = p_pool.tile([P, MW], f32, tag="ps",
                                 name=f"ps{nt}_{mh}")

                def mm16(kt, start, stop):
                    nc.tensor.matmul(
                        ps[:], lhsT=wq16[:, kt, ns], rhs=xq16[:, kt, ms],
                        start=start, stop=stop)

                def mm8(kp, start, stop):
                    nc.tensor.matmul(
                        ps[:],
                        lhsT=wq8[:, 2 * kp:2 * kp + 2, ns],
                        rhs=xq8[:, 2 * kp:2 * kp + 2, ms],
                        start=start, stop=stop, perf_mode=DR)

                if mh == 0:
                    for kt in range(KT16):
                        mm16(kt, kt == 0, False)
                    for kp in range(KT8 // 2):
                        mm8(kp, False, kp == KT8 // 2 - 1)
                else:
                    # fp8 X half arrives first in the mh=1 window
                    for kp in range(KT8 // 2):
                        mm8(kp, kp == 0, False)
                    for kt in range(KT16):
                        mm16(kt, False, kt == KT16 - 1)

                osb = o_pool.tile([P, MW], f16, tag="osb",
                                  name=f"osb{nt}_{mh}")
                nc.scalar.activation(
                    out=osb[:], in_=ps[:],
                    func=mybir.ActivationFunctionType.Copy, scale=2.0)
                nc.sync.dma_start(out=out3[:, nt, ms], in_=osb[:])

    _split_multiwait_instructions(nc)
    return nc


_NC_CACHE = None


def _get_nc() -> bass.Bass:
    global _NC_CACHE
    if _NC_CACHE is None:
        _NC_CACHE = _build_nc()
    return _NC_CACHE


def _run(inputs: dict, trace: bool = False, **kw):
    X = np.asarray(inputs["X"], dtype=np.float32)
    W = np.asarray(inputs["W"], dtype=np.float32)
    assert X.shape == (M_FULL, K) and W.shape == (N, K)

    XT = np.ascontiguousarray(X.T)            # [K, M_FULL]
    WT = np.ascontiguousarray(W.T)            # [K, N]
    in_maps = [
        {"xt": np.ascontiguousarray(XT[:, c * M:(c + 1) * M]), "wt": WT}
        for c in range(N_CORES)
    ]
    res = run_bass_kernel_spmd(
        _get_nc(), in_maps, list(range(N_CORES)), trace=trace, **kw)
    out = np.concatenate(
        [np.asarray(res.results[c]["out"]).T for c in range(N_CORES)],
        axis=0).astype(np.float32)
    return out, res


def kernel(X: np.ndarray, W: np.ndarray) -> np.ndarray:
    out, _ = _run({"X": X, "W": W})
    return out


# revision 3
# speedup vs baseline: 1.4181x; 1.1792x over previous
"""BinarizedFCLayer forward on 8 trn2 NeuronCores.

    out = X @ sign(W).T      X: [8192, 2048] f32, W: [2048, 2048] f32
                             sign(w) = +1 if w >= 0 else -1

Strategy
--------
Data-parallel over the batch dim of X: core c computes rows
[c*1024, (c+1)*1024) of the output; W is replicated. W-stationary /
out^T orientation: psum[n=128, m=512] = sum_k Sq^T[k, nt] * X^T[k, mh].

Mixed-precision contraction split beats the 1-column/cycle PE limit:
  * k-tiles 0..7  (K=1024): fp16 matmuls (X cast f32->f16 in DMA).
  * k-tiles 8..15 (K=1024): fp8e4 (e4m3) matmuls in DoubleRow perf
    mode -- two k-tiles per matmul at the same 512-cycle issue cost
    (verified exact + full-rate on HW). e4m3 X quantization costs
    ~2.6e-2 rel err on its half of K -> total ~1.87e-2 < 2e-2 gate.
  * W: cast f32->bf16 in DMA (sign-preserving), binarized on DVE in
    one pass to +-0.5 ((w>=0)-0.5) as fp16 (k 0..7) and fp8 (k 8..15);
    the missing x2 rides the PSUM-consumer ops (scale=2.0).
PE stream: 512 -> 384 matmul slots of 512 columns.

Two-phase stream, paced to the ~0.36 MiB/us single-queue HBM delivery
(fp8 k's consume f32 input bytes 2x faster per slot, so all-fp16
first keeps consumption ~matched to delivery):
  A: 32 units (nt, mh) x 8 fp16 matmuls -> ACT copy (x2) -> osb fp16.
  B: same units x 4 DoubleRow matmuls -> DVE scalar_tensor_tensor
     osb += 2*psum. Stores follow on the sync queue behind a gate DMA
     (keeps output writes off the HBM read stream).
Group order (chunk c = 4 n-tiles, interleaving m-halves so the late
X^T m-half pieces are only needed mid-phase):
  (c0,m0)(c1,m0)(c0,m1)(c2,m0)(c1,m1)(c3,m0)(c2,m1)(c3,m1)
W is passed host-side in 512-column-chunk-major layout so every W DMA
piece reads 2 KiB contiguous per descriptor row (512 B rows measurably
drop HBM read throughput ~20%). Warm-up matmuls bridge the DMA
prologue so the real stream starts warm at ~t0=19.5 us.

The walrus build allows at most ONE sync wait per instruction, so a
post-pass splits any multi-wait instruction into single-wait NoOps on
the same engine placed immediately before it.
"""

import numpy as np

try:
    import concourse.bass as bass
except ImportError:  # harness may run from a bare directory
    import sys
    for p in ("/opt/trn_rl_repo", "/root/.axon_site/_ro/trn_rl_repo"):
        if p not in sys.path:
            sys.path.append(p)
    import concourse.bass as bass

import concourse.mybir as mybir
from concourse.tile import TileContext
from concourse.bass_utils import run_bass_kernel_spmd

P = 128
N_CORES = 8
M_FULL, K, N = 8192, 2048, 2048
M = M_FULL // N_CORES          # 1024 rows of X per core
KT = K // P                    # 16 k-tiles
KT16 = 8                       # k-tiles 0..7  -> fp16 path
KT8 = 8                        # k-tiles 8..15 -> fp8 DoubleRow path
NC = 4                         # 4 n-chunks of 512 (4 n-tiles each)
MH = 2                         # 2 m-halves of 512
MW = 512
N_WARM = 150                   # dummy matmuls bridging preamble -> first data

f32 = mybir.dt.float32
f16 = mybir.dt.float16
bf16 = mybir.dt.bfloat16
f8e4 = mybir.dt.float8e4
DR = mybir.MatmulPerfMode.DoubleRow

# (chunk, m-half) group order for both phases: X m-half-1 pieces are
# needed only from group 2 on; W chunks are needed one group apart.
GROUPS = [(0, 0), (1, 0), (0, 1), (2, 0), (1, 1), (3, 0), (2, 1), (3, 1)]

# Input DMA piece order (gpsimd/SWDGE queue order == transfer order).
# W16/W8 pieces: wraw[:, ktlo:kthi, c*512:(c+1)*512]  (1 MiB / 4 kt)
# X16/X8 pieces: xq16/xq8[:, ktlo:kthi, mh*512:+512]  (1 MiB / 2 kt eq)
PIECE_ORDER = [
    ('W16', 0, 0, 4), ('X16', 0, 0, 4), ('W16', 0, 4, 8), ('X16', 0, 4, 8),
    ('W16', 1, 0, 4), ('W16', 1, 4, 8),
    ('X16', 1, 0, 4), ('X16', 1, 4, 8),
    ('W16', 2, 0, 8), ('W16', 3, 0, 8),
    ('X8', 0, 0, 8), ('W8', 0, 0, 8), ('W8', 1, 0, 8),
    ('X8', 1, 0, 8), ('W8', 2, 0, 8), ('W8', 3, 0, 8),
]


def _split_multiwait_instructions(nc: bass.Bass) -> int:
    """walrus codegen rejects >1 sync wait per instruction. Hoist extra waits
    onto fresh single-wait NoOps on the same engine right before the
    offending instruction (same-engine sequential waits are equivalent)."""
    n_split = 0
    for fn in nc.m.functions:
        for blk in fn.blocks:
            out = []
            for inst in blk.instructions:
                si = inst.sync_info
                if si is not None and si.on_wait and len(si.on_wait) > 1:
                    waits = list(si.on_wait)
                    for j, w in enumerate(waits[:-1]):
                        nop = mybir.InstNoOp(
                            name=f"{inst.name}_wsplit{j}", ins=[], outs=[])
                        nop.engine = inst.engine
                        nop.sync_info = mybir.SyncInfo(
                            on_wait=[w], on_update=[])
                        out.append(nop)
                        n_split += 1
                    inst.sync_info = mybir.SyncInfo(
                        on_wait=[waits[-1]],
                        on_update=list(si.on_update or []))
                out.append(inst)
            blk.instructions[:] = out
    return n_split


def _build_nc() -> bass.Bass:
    nc = bass.Bass()
    xt = nc.declare_dram_parameter("xt", [K, M], f32, isOutput=False)
    # W^T in 512-col-chunk-major layout: row (c*K + k) = W^T[k, c*512:+512]
    wt = nc.declare_dram_parameter("wt", [NC * K, MW], f32, isOutput=False)
    out = nc.declare_dram_parameter("out", [N, M], f16, isOutput=True)

    xt3 = xt[:].rearrange("(kt p) m -> p kt m", p=P)    # [128, 16, 1024]
    wt4 = wt[:].rearrange("(c kt p) n -> p c kt n",
                          c=NC, p=P)                    # [128, 4, 16, 512]
    out3 = out[:].rearrange("(nt p) m -> p nt m", p=P)  # [128, 16, 1024]

    with TileContext(nc) as tc:
        with (
            tc.tile_pool(name="resident", bufs=1) as res_pool,
            tc.tile_pool(name="osb", bufs=32) as o_pool,
            tc.tile_pool(name="gate", bufs=1) as g_pool,
            tc.tile_pool(name="psum", bufs=8, space="PSUM") as p_pool,
            tc.tile_pool(name="warm", bufs=1) as warm_pool,
        ):
            xq16 = res_pool.tile([P, KT16, M], f16, tag="xq16", name="xq16")
            xq8 = res_pool.tile([P, KT8, M], f8e4, tag="xq8", name="xq8")
            wraw = res_pool.tile([P, KT, N], bf16, tag="wraw", name="wraw")
            wq16 = res_pool.tile([P, KT16, N], f16, tag="wq16", name="wq16")
            wq8 = res_pool.tile([P, KT8, N], f8e4, tag="wq8", name="wq8")

            # PE warm-up first in each queue: memset leads the DVE queue,
            # dummy matmuls lead the PE queue, bridging the DMA prologue
            # and holding the HAM clock gate at 8/8 for the real stream.
            wsrc = warm_pool.tile([P, P], f16, tag="wsrc", name="wsrc")
            nc.vector.memset(wsrc[:], 0.0)
            wps = p_pool.tile([P, MW], f32, tag="ps", name="wps")
            for _ in range(N_WARM):
                nc.tensor.matmul(wps[:, :P], lhsT=wsrc[:], rhs=wsrc[:],
                                 start=True, stop=True)

            # Input pieces on the SWDGE queue; every W piece is binarized
            # on DVE as soon as it lands ((w >= 0) - 0.5 -> +-0.5).
            for kind, idx, klo, khi in PIECE_ORDER:
                if kind in ('W16', 'W8'):
                    ko = 0 if kind == 'W16' else KT16
                    ns = slice(idx * MW, (idx + 1) * MW)
                    nc.gpsimd.dma_start(
                        out=wraw[:, ko + klo:ko + khi, ns],
                        in_=wt4[:, idx, ko + klo:ko + khi, :])
                    dst = wq16 if kind == 'W16' else wq8
                    nc.vector.tensor_scalar(
                        dst[:, klo:khi, ns], wraw[:, ko + klo:ko + khi, ns],
                        0.0, 0.5,
                        mybir.AluOpType.is_ge, mybir.AluOpType.subtract)
                elif kind == 'X16':
                    ms = slice(idx * MW, (idx + 1) * MW)
                    nc.gpsimd.dma_start(out=xq16[:, klo:khi, ms],
                                        in_=xt3[:, klo:khi, ms])
                else:  # X8
                    ms = slice(idx * MW, (idx + 1) * MW)
                    nc.gpsimd.dma_start(out=xq8[:, klo:khi, ms],
                                        in_=xt3[:, KT16 + klo:KT16 + khi, ms])

            # Store gate: a tiny sync-queue DMA that reads the tail of the
            # last input piece; stores queue behind it in sync-FIFO order.
            gsc = g_pool.tile([1, 4], bf16, tag="gate", name="gate")
            nc.sync.dma_start(
                out=gsc[:], in_=wraw[0:1, KT - 1:KT, N - 4:N])

            # Phase A: fp16 half of K, kt-outer within each 4-unit group.
            osbs = {}
            for c, mh in GROUPS:
                ms = slice(mh * MW, (mh + 1) * MW)
                pss = [p_pool.tile([P, MW], f32, tag="ps",
                                   name=f"psA{c}_{mh}_{j}") for j in range(4)]
                for kt in range(KT16):
                    for j in range(4):
                        nt = 4 * c + j
                        nc.tensor.matmul(
                            pss[j][:],
                            lhsT=wq16[:, kt, nt * P:(nt + 1) * P],
                            rhs=xq16[:, kt, ms],
                            start=(kt == 0), stop=(kt == KT16 - 1))
                for j in range(4):
                    nt = 4 * c + j
                    osb = o_pool.tile([P, MW], f16, tag="osb",
                                      name=f"osb{nt}_{mh}")
                    osbs[(nt, mh)] = osb
                    nc.scalar.activation(
                        out=osb[:], in_=pss[j][:],
                        func=mybir.ActivationFunctionType.Copy, scale=2.0)

            # Phase B: fp8 DoubleRow half of K; osb += 2*psum on DVE;
            # store right after on the gated sync queue.
            for c, mh in GROUPS:
                ms = slice(mh * MW, (mh + 1) * MW)
                pss = [p_pool.tile([P, MW], f32, tag="ps",
                                   name=f"psB{c}_{mh}_{j}") for j in range(4)]
                for kp in range(KT8 // 2):
                    for j in range(4):
                        nt = 4 * c + j
                        nc.tensor.matmul(
                            pss[j][:],
                            lhsT=wq8[:, 2 * kp:2 * kp + 2, nt * P:(nt + 1) * P],
                            rhs=xq8[:, 2 * kp:2 * kp + 2, ms],
                            start=(kp == 0), stop=(kp == KT8 // 2 - 1),
                            perf_mode=DR)
                for j in range(4):
                    nt = 4 * c + j
                    osb = osbs[(nt, mh)]
                    nc.vector.scalar_tensor_tensor(
                        out=osb[:], in0=pss[j][:], scalar=2.0, in1=osb[:],
                        op0=mybir.AluOpType.mult, op1=mybir.AluOpType.add)
                    nc.sync.dma_start(out=out3[:, nt, ms], in_=osb[:])

    _split_multiwait_instructions(nc)
    return nc


_NC_CACHE = None


def _get_nc() -> bass.Bass:
    global _NC_CACHE
    if _NC_CACHE is None:
        _NC_CACHE = _build_nc()
    return _NC_CACHE


def _run(inputs: dict, trace: bool = False, **kw):
    X = np.asarray(inputs["X"], dtype=np.float32)
    W = np.asarray(inputs["W"], dtype=np.float32)
    assert X.shape == (M_FULL, K) and W.shape == (N, K)

    XT = np.ascontiguousarray(X.T)            # [K, M_FULL]
    WT = np.ascontiguousarray(W.T)            # [K, N]
    # 512-col-chunk-major W^T so each W DMA piece reads 2 KiB rows
    WTr = np.ascontiguousarray(
        WT.reshape(K, NC, MW).transpose(1, 0, 2).reshape(NC * K, MW))
    in_maps = [
        {"xt": np.ascontiguousarray(XT[:, c * M:(c + 1) * M]), "wt": WTr}
        for c in range(N_CORES)
    ]
    res = run_bass_kernel_spmd(
        _get_nc(), in_maps, list(range(N_CORES)), trace=trace, **kw)
    out = np.concatenate(
        [np.asarray(res.results[c]["out"]).T for c in range(N_CORES)],
        axis=0).astype(np.float32)
    return out, res


def kernel(X: np.ndarray, W: np.ndarray) -> np.ndarray:
    out, _ = _run({"X": X, "W": W})
    return out


# revision 4
# speedup vs baseline: 1.5176x; 1.0702x over previous
"""BinarizedFCLayer forward on 8 trn2 NeuronCores.

    out = X @ sign(W).T      X: [8192, 2048] f32, W: [2048, 2048] f32
                             sign(w) = +1 if w >= 0 else -1

Strategy
--------
Data-parallel over the batch dim of X: core c computes rows
[c*1024, (c+1)*1024) of the output; W is replicated. W-stationary /
out^T orientation: psum[n=128, m=512] = sum_k Sq^T[k, nt] * X^T[k, mh].

Mixed-precision contraction split beats the 1-column/cycle PE limit:
  * k-tiles 0..7  (K=1024): fp16 matmuls (X cast f32->f16 in DMA).
  * k-tiles 8..15 (K=1024): fp8e4 (e4m3) matmuls in DoubleRow perf
    mode -- two k-tiles per matmul at the same 512-cycle issue cost
    (verified exact + full-rate on HW). e4m3 X quantization costs
    ~2.6e-2 rel err on its half of K -> total ~1.87e-2 < 2e-2 gate.
  * W: cast f32->bf16 in DMA (sign-preserving), binarized on DVE in
    one pass to +-0.5 ((w>=0)-0.5) as fp16 (k 0..7) and fp8 (k 8..15);
    the missing x2 rides the PSUM-consumer ops (scale=2.0).
PE stream: 512 -> 384 matmul slots of 512 columns.

Two-phase stream, paced to the ~0.36 MiB/us single-queue HBM delivery
(fp8 k's consume f32 input bytes 2x faster per slot, so all-fp16
first keeps consumption ~matched to delivery):
  A: 32 units (nt, mh) x 8 fp16 matmuls -> ACT copy (x2) -> osb fp16.
  B: same units x 4 DoubleRow matmuls -> DVE scalar_tensor_tensor
     osb += 2*psum. Stores follow on the sync queue behind a gate DMA
     (keeps output writes off the HBM read stream).
Group order (chunk c = 4 n-tiles, interleaving m-halves so the late
X^T m-half pieces are only needed mid-phase):
  (c0,m0)(c1,m0)(c0,m1)(c2,m0)(c1,m1)(c3,m0)(c2,m1)(c3,m1)
W is passed host-side in 512-column-chunk-major layout so every W DMA
piece reads 2 KiB contiguous per descriptor row (512 B rows measurably
drop HBM read throughput ~20%). Warm-up matmuls bridge the DMA
prologue so the real stream starts warm at ~t0=19.5 us.

The walrus build allows at most ONE sync wait per instruction, so a
post-pass splits any multi-wait instruction into single-wait NoOps on
the same engine placed immediately before it.
"""

import numpy as np

try:
    import concourse.bass as bass
except ImportError:  # harness may run from a bare directory
    import sys
    for p in ("/opt/trn_rl_repo", "/root/.axon_site/_ro/trn_rl_repo"):
        if p not in sys.path:
            sys.path.append(p)
    import concourse.bass as bass

import concourse.mybir as mybir
from concourse.tile import TileContext
from concourse.bass_utils import run_bass_kernel_spmd

P = 128
N_CORES = 8
M_FULL, K, N = 8192, 2048, 2048
M = M_FULL // N_CORES          # 1024 rows of X per core
KT = K // P                    # 16 k-tiles
KT16 = 8                       # k-tiles 0..7  -> fp16 path
KT8 = 8                        # k-tiles 8..15 -> fp8 DoubleRow path
NC = 4                         # 4 n-chunks of 512 (4 n-tiles each)
MH = 2                         # 2 m-halves of 512
MW = 512
N_WARM = 150                   # dummy matmuls bridging preamble -> first data

f32 = mybir.dt.float32
f16 = mybir.dt.float16
bf16 = mybir.dt.bfloat16
f8e4 = mybir.dt.float8e4
DR = mybir.MatmulPerfMode.DoubleRow

# (chunk, m-half) group order for both phases: X m-half-1 pieces are
# needed only from group 2 on; W chunks are needed one group apart.
GROUPS = [(0, 0), (1, 0), (0, 1), (2, 0), (1, 1), (3, 0), (2, 1), (3, 1)]

# Input DMA piece order (gpsimd/SWDGE queue order == transfer order).
# W16/W8 pieces: wraw[:, ktlo:kthi, c*512:(c+1)*512]  (1 MiB / 4 kt)
# X16/X8 pieces: xq16/xq8[:, ktlo:kthi, mh*512:+512]  (1 MiB / 2 kt eq)
PIECE_ORDER = [
    ('W16', 0, 0, 4), ('X16', 0, 0, 4), ('W16', 0, 4, 8), ('X16', 0, 4, 8),
    ('W16', 1, 0, 4), ('W16', 1, 4, 8),
    ('X16', 1, 0, 4), ('X16', 1, 4, 8),
    ('W16', 2, 0, 8), ('W16', 3, 0, 8),
    ('W8', 0, 0, 8), ('X8', 0, 0, 8), ('W8', 1, 0, 8),
    ('W8', 2, 0, 8), ('W8', 3, 0, 8), ('X8', 1, 0, 8),
]


def _split_multiwait_instructions(nc: bass.Bass) -> int:
    """walrus codegen rejects >1 sync wait per instruction. Hoist extra waits
    onto fresh single-wait NoOps on the same engine right before the
    offending instruction (same-engine sequential waits are equivalent)."""
    n_split = 0
    for fn in nc.m.functions:
        for blk in fn.blocks:
            out = []
            for inst in blk.instructions:
                si = inst.sync_info
                if si is not None and si.on_wait and len(si.on_wait) > 1:
                    waits = list(si.on_wait)
                    for j, w in enumerate(waits[:-1]):
                        nop = mybir.InstNoOp(
                            name=f"{inst.name}_wsplit{j}", ins=[], outs=[])
                        nop.engine = inst.engine
                        nop.sync_info = mybir.SyncInfo(
                            on_wait=[w], on_update=[])
                        out.append(nop)
                        n_split += 1
                    inst.sync_info = mybir.SyncInfo(
                        on_wait=[waits[-1]],
                        on_update=list(si.on_update or []))
                out.append(inst)
            blk.instructions[:] = out
    return n_split


def _build_nc() -> bass.Bass:
    nc = bass.Bass()
    xt = nc.declare_dram_parameter("xt", [K, M], f32, isOutput=False)
    # W^T in 512-col-chunk-major layout: row (c*K + k) = W^T[k, c*512:+512]
    wt = nc.declare_dram_parameter("wt", [NC * K, MW], f32, isOutput=False)
    out = nc.declare_dram_parameter("out", [N, M], f16, isOutput=True)

    xt3 = xt[:].rearrange("(kt p) m -> p kt m", p=P)    # [128, 16, 1024]
    wt4 = wt[:].rearrange("(c kt p) n -> p c kt n",
                          c=NC, p=P)                    # [128, 4, 16, 512]
    out3 = out[:].rearrange("(nt p) m -> p nt m", p=P)  # [128, 16, 1024]

    with TileContext(nc) as tc:
        with (
            tc.tile_pool(name="resident", bufs=1) as res_pool,
            tc.tile_pool(name="osb", bufs=32) as o_pool,
            tc.tile_pool(name="gate", bufs=1) as g_pool,
            tc.tile_pool(name="psum", bufs=8, space="PSUM") as p_pool,
            tc.tile_pool(name="warm", bufs=1) as warm_pool,
        ):
            xq16 = res_pool.tile([P, KT16, M], f16, tag="xq16", name="xq16")
            xq8 = res_pool.tile([P, KT8, M], f8e4, tag="xq8", name="xq8")
            wraw = res_pool.tile([P, KT, N], bf16, tag="wraw", name="wraw")
            wq16 = res_pool.tile([P, KT16, N], f16, tag="wq16", name="wq16")
            wq8 = res_pool.tile([P, KT8, N], f8e4, tag="wq8", name="wq8")

            # PE warm-up first in each queue: memset leads the DVE queue,
            # dummy matmuls lead the PE queue, bridging the DMA prologue
            # and holding the HAM clock gate at 8/8 for the real stream.
            wsrc = warm_pool.tile([P, P], f16, tag="wsrc", name="wsrc")
            nc.vector.memset(wsrc[:], 0.0)
            wps = p_pool.tile([P, MW], f32, tag="ps", name="wps")
            for _ in range(N_WARM):
                nc.tensor.matmul(wps[:, :P], lhsT=wsrc[:], rhs=wsrc[:],
                                 start=True, stop=True)

            # Input pieces on the SWDGE queue; every W piece is binarized
            # on DVE as soon as it lands ((w >= 0) - 0.5 -> +-0.5).
            for kind, idx, klo, khi in PIECE_ORDER:
                if kind in ('W16', 'W8'):
                    ko = 0 if kind == 'W16' else KT16
                    ns = slice(idx * MW, (idx + 1) * MW)
                    nc.gpsimd.dma_start(
                        out=wraw[:, ko + klo:ko + khi, ns],
                        in_=wt4[:, idx, ko + klo:ko + khi, :])
                    dst = wq16 if kind == 'W16' else wq8
                    nc.vector.tensor_scalar(
                        dst[:, klo:khi, ns], wraw[:, ko + klo:ko + khi, ns],
                        0.0, 0.5,
                        mybir.AluOpType.is_ge, mybir.AluOpType.subtract)
                elif kind == 'X16':
                    ms = slice(idx * MW, (idx + 1) * MW)
                    nc.gpsimd.dma_start(out=xq16[:, klo:khi, ms],
                                        in_=xt3[:, klo:khi, ms])
                else:  # X8
                    ms = slice(idx * MW, (idx + 1) * MW)
                    nc.gpsimd.dma_start(out=xq8[:, klo:khi, ms],
                                        in_=xt3[:, KT16 + klo:KT16 + khi, ms])

            # Store gate: a tiny sync-queue DMA that reads the tail of the
            # last input piece; stores queue behind it in sync-FIFO order.
            gsc = g_pool.tile([1, 4], bf16, tag="gate", name="gate")
            nc.sync.dma_start(
                out=gsc[:], in_=wraw[0:1, KT - 1:KT, N - 4:N])

            # Phase A: fp16 half of K, kt-outer within each 4-unit group.
            osbs = {}
            for c, mh in GROUPS:
                ms = slice(mh * MW, (mh + 1) * MW)
                pss = [p_pool.tile([P, MW], f32, tag="ps",
                                   name=f"psA{c}_{mh}_{j}") for j in range(4)]
                for kt in range(KT16):
                    for j in range(4):
                        nt = 4 * c + j
                        nc.tensor.matmul(
                            pss[j][:],
                            lhsT=wq16[:, kt, nt * P:(nt + 1) * P],
                            rhs=xq16[:, kt, ms],
                            start=(kt == 0), stop=(kt == KT16 - 1))
                for j in range(4):
                    nt = 4 * c + j
                    osb = o_pool.tile([P, MW], f16, tag="osb",
                                      name=f"osb{nt}_{mh}")
                    osbs[(nt, mh)] = osb
                    nc.scalar.activation(
                        out=osb[:], in_=pss[j][:],
                        func=mybir.ActivationFunctionType.Copy, scale=2.0)

            # Phase B: fp8 DoubleRow half of K; osb += 2*psum on DVE;
            # store right after on the gated sync queue. All-mh0 groups
            # first: the fp8 X m-half-1 piece is the last to arrive.
            for c, mh in [(0, 0), (1, 0), (2, 0), (3, 0),
                          (0, 1), (1, 1), (2, 1), (3, 1)]:
                ms = slice(mh * MW, (mh + 1) * MW)
                pss = [p_pool.tile([P, MW], f32, tag="ps",
                                   name=f"psB{c}_{mh}_{j}") for j in range(4)]
                for kp in range(KT8 // 2):
                    for j in range(4):
                        nt = 4 * c + j
                        nc.tensor.matmul(
                            pss[j][:],
                            lhsT=wq8[:, 2 * kp:2 * kp + 2, nt * P:(nt + 1) * P],
                            rhs=xq8[:, 2 * kp:2 * kp + 2, ms],
                            start=(kp == 0), stop=(kp == KT8 // 2 - 1),
                            perf_mode=DR)
                for j in range(4):
                    nt = 4 * c + j
                    osb = osbs[(nt, mh)]
                    nc.vector.scalar_tensor_tensor(
                        out=osb[:], in0=pss[j][:], scalar=2.0, in1=osb[:],
                        op0=mybir.AluOpType.mult, op1=mybir.AluOpType.add)
                    nc.sync.dma_start(out=out3[:, nt, ms], in_=osb[:])

    _split_multiwait_instructions(nc)
    return nc


_NC_CACHE = None


def _get_nc() -> bass.Bass:
    global _NC_CACHE
    if _NC_CACHE is None:
        _NC_CACHE = _build_nc()
    return _NC_CACHE


def _run(inputs: dict, trace: bool = False, **kw):
    X = np.asarray(inputs["X"], dtype=np.float32)
    W = np.asarray(inputs["W"], dtype=np.float32)
    assert X.shape == (M_FULL, K) and W.shape == (N, K)

    XT = np.ascontiguousarray(X.T)            # [K, M_FULL]
    WT = np.ascontiguousarray(W.T)            # [K, N]
    # 512-col-chunk-major W^T so each W DMA piece reads 2 KiB rows
    WTr = np.ascontiguousarray(
        WT.reshape(K, NC, MW).transpose(1, 0, 2).reshape(NC * K, MW))
    in_maps = [
        {"xt": np.ascontiguousarray(XT[:, c * M:(c + 1) * M]), "wt": WTr}
        for c in range(N_CORES)
    ]
    res = run_bass_kernel_spmd(
        _get_nc(), in_maps, list(range(N_CORES)), trace=trace, **kw)
    out = np.concatenate(
        [np.asarray(res.results[c]["out"]).T for c in range(N_CORES)],
        axis=0).astype(np.float32)
    return out, res


def kernel(X: np.ndarray, W: np.ndarray) -> np.ndarray:
    out, _ = _run({"X": X, "W": W})
    return out
